# revision 2
# baseline (speedup 1.0000x reference)
"""Causal multi-head attention (QKV projection + softmax(QK^T)V) on 8 TRN2 NeuronCores.

Problem: x[4,2048,1024] @ W_qkv[1024,3072] + b_qkv -> 16-head causal attention -> [4,2048,1024].

Sharding: core i = (batch bi=i//2, head-group hg=i%2). Each core handles 1 batch x 8 heads,
fully data/tensor-parallel (no collectives). Host pre-arranges per-core inputs (all matmul
operands fp16; accumulation f32 in PSUM):
  - x passed pre-transposed [1024, 2048] so the contraction dim lands on partitions with
    plain contiguous DMAs (no on-device transposes anywhere).
  - wqk [1024,1024] pair-major (pair p: Q cols at 256p, K at 256p+128), head-PAIR-stacked
    (64+64 rows) so QKV^T matmul output chunks are directly the [hd, n] stacked layout the
    attention stage consumes.
  - wv [1024,520]: V columns with per-head stride 65; col 65h+64 is a zero column, and
    the replicated bias tile bv has 1.0 there, so the "ones column" that makes the PV
    matmul accumulate softmax denominators (and b_v itself) ride the DVE PSUM->SBUF
    drain as a tensor_add -- no bias matmuls at all.
Device pipeline per core:
  QKV^T matmuls (Q^T pair-stacked, K^T pair-packed: head h occupies rows 64*(h%2) of its
  pair's slab, S^T runs as a 64-contraction matmul at base_partition 64*(h%2), which also
  row-tiles the two heads onto disjoint PE array row-groups) -> S^T = K Q^T per key-chunk
  with causal column trimming -> one ScalarE Exp(scale=1/8) per 2-chunk group, PSUM->SBUF
  fp16 = P^T -> causal tri-mask multiply on the 128x128 diagonal blocks only (DVE) ->
  PV matmuls accumulate [q, 64 cols + denominator] per q-block (both heads packed in one
  PSUM bank) -> reciprocal (DVE) * scale (DVE late / ScalarE early) epilogue ->
  DMA out [2048, 512] f32.
Scheduling: ScalarE exp (~160us) and TensorE (~190us) must overlap near-perfectly.
 - A dozen warm-up matmuls on memset scratch run first so the PE HAM clock-gate reaches
   K=8/8 (2.4 GHz) before real work; without them the DMA-gated trickle start keeps the
   PE at 1.2 GHz for the first ~20us of real matmuls.
 - Input DMA is spread over three rings so triggers issue in parallel: sync ring carries
   the latency-critical x-stripe-0 + wqk-pair-0 pieces per-chunk; the scalar ring (idle
   before the first softmax) carries batched x stripes 1-3; gpsimd SWDGE carries the
   rest (wqk pairs 1-3, wv, biases) -- ScalarE's instruction stream stays pure exps by
   the time softmax starts.
 - Attention runs stripe-major across head-pairs (t-major rounds). QKV matmul tiles are
   distributed across blocks by a build-time reverse-greedy quota scheduler: each block's
   spare PE time (ACT cost minus S^T/PV cost) is filled latest-first subject to each
   tile's consumer deadline, so the ACT-heavy late rounds keep the PE dense instead of
   exhausting the filler early.
 - Each stripe's PV matmuls are deferred into the next block's S^T/exp loop (inlined
   per-diagonal-chunk for the final block so the tail doesn't serialize).
"""

import numpy as np

import concourse.bass as bass
import concourse.tile as tile
from concourse import bacc, mybir
from concourse import bass_utils

F16 = mybir.dt.float16
F32 = mybir.dt.float32

B, N, D = 4, 2048, 1024
H = 16  # global heads
HD = 64
HL = 8  # heads per core
N_CORES = 8
P = 128
NT = N // P  # 16 token tiles
KC = D // P  # 8 contraction chunks
VW = HL * (HD + 1)  # 520
VH = VW // 2  # 260

_cache = {}


def _build():
    nc = bacc.Bacc("TRN2", target_bir_lowering=False, debug=False)

    x_d = nc.dram_tensor("x", [D, N], F16, kind="ExternalInput").ap()  # x^T, host-transposed
    wqk_d = nc.dram_tensor("wqk", [D, 1024], F16, kind="ExternalInput").ap()
    wv_d = nc.dram_tensor("wv", [D, VW], F16, kind="ExternalInput").ap()
    bqk_d = nc.dram_tensor("bqk", [P, 8], F32, kind="ExternalInput").ap()
    bv_d = nc.dram_tensor("bv", [P, VW], F16, kind="ExternalInput").ap()
    tri_d = nc.dram_tensor("tri", [P, P], F16, kind="ExternalInput").ap()
    out_d = nc.dram_tensor("out", [N, HL * HD], F32, kind="ExternalOutput").ap()

    wqk_r = wqk_d.rearrange("(k p) n -> p k n", p=P)
    wv_r = wv_d.rearrange("(k p) n -> p k n", p=P)
    x_r = x_d.rearrange("(k p) n -> p k n", p=P)

    with tile.TileContext(nc) as tc:
        with (
            tc.tile_pool(name="const", bufs=1) as cpool,
            tc.tile_pool(name="pt", bufs=2) as ptpool,
            tc.tile_pool(name="opair", bufs=6) as oppool,
            tc.tile_pool(name="misc", bufs=6) as mpool,
            tc.tile_pool(name="ps_mm", bufs=2, space="PSUM") as ps_mm,
            tc.tile_pool(name="ps_s", bufs=2, space="PSUM") as ps_s,
            tc.tile_pool(name="ps_o", bufs=2, space="PSUM") as ps_o,
        ):
            # ---- constants / inputs to SBUF ----
            xt_sb = cpool.tile([P, KC, N], F16, name="xt_sb")  # x^T, 8 chunks of [128, 2048]
            wqk_sb = cpool.tile([P, KC, 1024], F16, name="wqk_sb")
            wv_sb = cpool.tile([P, KC, VW], F16, name="wv_sb")
            bqk_sb = cpool.tile([P, 8], F32, name="bqk_sb")
            bv_sb = cpool.tile([P, VW], F16, name="bv_sb")  # b_v (+ones col) replicated
            tri_sb = cpool.tile([P, P], F16, name="tri_sb")
            qt_sb = cpool.tile([P, 4, N], F16, name="qt_sb")  # Q^T pair-stacked
            # K^T pair-packed: pair pr's slab holds head 2pr K^T in rows 0:64 and head
            # 2pr+1 in rows 64:128. S^T runs as a 64-row contraction at base_partition
            # 64*hh -- no zero padding, no memset, and the two heads land on disjoint
            # PE row-groups so their LDWEIGHTS/MATMULs overlap in the array.
            kt_sb = cpool.tile([P, 4, N], F16, name="kt_sb")
            v_sb = cpool.tile([P, NT, VW], F16, name="v_sb")
            wu_sb = cpool.tile([P, 640], F16, name="wu_sb")  # PE warm-up scratch

            # PE warm-up: the HAM clock gate un-throttles (1.2 -> 2.4 GHz) only after
            # ~3.4us of sustained PE activity. Burn that window on scratch matmuls while
            # the input DMAs are in flight, so real matmuls start at full clock.
            nc.gpsimd.memset(wu_sb[:], 0.0)
            psw = ps_mm.tile([P, 512], F32, tag="mm", name="ps_warm")
            for i in range(12):
                nc.tensor.matmul(
                    psw[:],
                    lhsT=wu_sb[:, 0:P],
                    rhs=wu_sb[:, P : P + 512],
                    start=(i == 0),
                    stop=(i == 11),
                )

            # Preload the exp table set (~2.7us) before the first real softmax exp, so
            # it doesn't pay ACT_TABLE_LOAD.
            warm = mpool.tile([1, 8], F32, tag="warm", name="warm")
            nc.gpsimd.memset(warm[:], 0.0)
            nc.scalar.activation(warm[:], warm[:], mybir.ActivationFunctionType.Exp)

            # ---- input DMA, spread across rings ----
            # sync ring: latency-critical x stripe-0 + wqk pair-0, per-chunk so the
            # first QKV tiles can start as soon as chunk 0 lands.
            for k in range(KC):
                nc.sync.dma_start(
                    xt_sb[:, k, 0:512], x_d[k * P : (k + 1) * P, 0:512]
                )
                nc.sync.dma_start(wqk_sb[:, k, 0:256], wqk_r[:, k, 0:256])
            # scalar ring (HWDGE; idle until the first exp ~14us): batched x stripes 1-3.
            for tt in (1, 2, 3):
                nc.scalar.dma_start(
                    xt_sb[:, :, tt * 512 : (tt + 1) * 512],
                    x_r[:, :, tt * 512 : (tt + 1) * 512],
                )
            # gpsimd SWDGE: everything else, ordered by first-use deadline.
            nc.gpsimd.dma_start(bqk_sb[:], bqk_d)
            nc.gpsimd.dma_start(wqk_sb[:, :, 256:512], wqk_r[:, :, 256:512])
            nc.gpsimd.dma_start(wv_sb[:, :, :], wv_r[:, :, :])
            nc.gpsimd.dma_start(tri_sb[:], tri_d)
            nc.gpsimd.dma_start(bv_sb[:], bv_d)
            nc.gpsimd.dma_start(wqk_sb[:, :, 512:768], wqk_r[:, :, 512:768])
            nc.gpsimd.dma_start(wqk_sb[:, :, 768:1024], wqk_r[:, :, 768:1024])

            done_qk = set()
            done_v = set()

            def emit_qk(c, tt):
                """QKV^T matmul tile for col-chunk c, token stripe tt."""
                if (c, tt) in done_qk:
                    return
                done_qk.add((c, tt))
                pr = c % 4
                pq = ps_mm.tile([P, 512], F32, tag="mm", name=f"pq_{c}_{tt}")
                col0 = 256 * (c % 4) + (0 if c < 4 else 128)
                for k in range(KC):
                    nc.tensor.matmul(
                        pq[:],
                        lhsT=wqk_sb[:, k, col0 : col0 + P],
                        rhs=xt_sb[:, k, tt * 512 : (tt + 1) * 512],
                        start=(k == 0),
                        stop=(k == KC - 1),
                    )

                def badd(out, in_, b):
                    nc.vector.tensor_scalar_add(out, in_, b)

                if c < 4:
                    badd(
                        qt_sb[:, pr, tt * 512 : (tt + 1) * 512], pq[:], bqk_sb[:, c : c + 1]
                    )
                else:
                    for hh in (0, 1):
                        rows = slice(64 * hh, 64 * hh + 64)
                        badd(
                            kt_sb[rows, pr, tt * 512 : (tt + 1) * 512],
                            pq[rows, :],
                            bqk_sb[rows, c : c + 1],
                        )

            def emit_v(j, half):
                """V (augmented) for token tile j, half (260 cols each)."""
                if (j, half) in done_v:
                    return
                done_v.add((j, half))
                pv = ps_mm.tile([P, VH], F32, tag="mm", name=f"pv_{j}_{half}")
                for k in range(KC):
                    nc.tensor.matmul(
                        pv[:],
                        lhsT=xt_sb[:, k, j * P : (j + 1) * P],
                        rhs=wv_sb[:, k, half * VH : (half + 1) * VH],
                        start=(k == 0),
                        stop=(k == KC - 1),
                    )
                # bias (and the denominator ones-column) ride the PSUM->SBUF drain
                nc.vector.tensor_add(
                    v_sb[:, j, half * VH : (half + 1) * VH],
                    pv[:],
                    bv_sb[:, half * VH : (half + 1) * VH],
                )

            # ---- build-time quota scheduler for QKV filler ----
            # blocks processed t-major: n = 4*t + p
            n_blocks = 16

            def act_cost(t):
                return (2 * t + 1) * 2086 + 1400

            def pe_s_cost(t):
                return 2 * (2048 * t + 1280) * 0.4167 + (8 * t + 8) * 15

            def pv_cost(t):
                return 2 * (16 * t + 10) * 39.0

            QK_COST = 8 * 512 * 0.4167 + 8 * 15
            V_COST = 8 * 260 * 0.4167 + 8 * 15

            # items: (kind, a, b, deadline_block, earliest_block, cost)
            items = []
            for tt in range(4):
                for pr in range(4):
                    if (pr, tt) == (0, 0):
                        continue  # prologue
                    dl = 4 * tt + pr
                    for c in (pr, 4 + pr):
                        items.append(["qk", c, tt, dl, max(0, tt - 1), QK_COST])
            for tj in range(4):
                for j in range(4 * tj, 4 * tj + 4):
                    for half in (0, 1):
                        dl = 4 * tj + 2 * half + 1
                        items.append(["v", j, half, dl, max(0, tj - 1), V_COST])

            cap = []
            for n in range(n_blocks):
                t = n // 4
                c = act_cost(t) - pe_s_cost(t)
                if n >= 1:
                    c -= pv_cost((n - 1) // 4)
                if n == n_blocks - 1:
                    c -= pv_cost(3)  # own inline PV
                cap.append(max(0.0, c))

            W = [[] for _ in range(n_blocks)]
            remaining = list(items)
            for n in range(n_blocks - 1, -1, -1):
                room = cap[n]
                # mandatory: last chance for items with deadline n+1
                musts = [it for it in remaining if it[3] == n + 1 and it[4] <= n]
                for it in musts:
                    W[n].append(it)
                    room -= it[5]
                    remaining.remove(it)
                elig = [it for it in remaining if it[3] > n + 1 and it[4] <= n]
                elig.sort(key=lambda it: -it[3])  # most-flexible first
                for it in elig:
                    if room <= 0:
                        break
                    W[n].append(it)
                    room -= it[5]
                    remaining.remove(it)
            # leftovers: emit as early as their earliest-block bound allows
            for it in remaining:
                W[it[4]].append(it)
            for n in range(n_blocks):
                W[n].sort(key=lambda it: it[3])  # urgent first within a block

            def emit_item(it):
                if it[0] == "qk":
                    emit_qk(it[1], it[2])
                else:
                    emit_v(it[1], it[2])

            state = {}

            def emit_pv_half(p, t, pt, r, hh, ctx, split_dma=False):
                """One head's PV chain for q-block i = 4t+r; epilogue+DMA after hh=1.
                Both heads' accumulators share one PSUM bank ([128, 2, 65])."""
                i = 4 * t + r
                hl = 2 * p + hh
                for j in range(i + 1):  # safety: deps normally already emitted
                    emit_v(j, hl // 4)
                if hh == 0:
                    ctx["opair"] = oppool.tile([P, P], F32, tag="op", name=f"op_{p}_{i}")
                    ctx["po"] = po = ps_o.tile([P, 2, 65], F32, tag="o", name=f"po_{p}_{i}")
                else:
                    po = ctx["po"]
                for j in range(i + 1):
                    nc.tensor.matmul(
                        po[:, hh, :],
                        lhsT=pt[:, hh, j, r * P : (r + 1) * P],
                        rhs=v_sb[:, j, 65 * hl : 65 * hl + 65],
                        start=(j == 0),
                        stop=(j == i),
                    )
                if hh == 0:
                    return
                opair = ctx["opair"]
                rc = mpool.tile([P, 2], F32, tag="rc", name=f"rc_{p}_{i}")
                nc.vector.reciprocal(rc[:], po[:, :, 64])
                for h2 in (0, 1):
                    if state.get("pos", 0) == 0:
                        # early rounds are PE/DVE-bound and ScalarE has slack
                        nc.scalar.mul(
                            opair[:, 64 * h2 : 64 * h2 + 64],
                            po[:, h2, 0:64],
                            rc[:, h2 : h2 + 1],
                        )
                    else:
                        nc.vector.tensor_scalar_mul(
                            opair[:, 64 * h2 : 64 * h2 + 64], po[:, h2, 0:64], rc[:, h2 : h2 + 1]
                        )
                    if split_dma:
                        nc.sync.dma_start(
                            out_d[i * P : (i + 1) * P, p * P + 64 * h2 : p * P + 64 * h2 + 64],
                            opair[:, 64 * h2 : 64 * h2 + 64],
                        )
                if not split_dma:
                    nc.sync.dma_start(out_d[i * P : (i + 1) * P, p * P : (p + 1) * P], opair[:])

            def emit_pv(p, t, pt, r, split_dma=False):
                ctx = {}
                emit_pv_half(p, t, pt, r, 0, ctx, split_dma)
                emit_pv_half(p, t, pt, r, 1, ctx, split_dma)

            # Prologue: the first block's own QK stripes so S^T (0,0) can start ASAP.
            emit_qk(0, 0)
            emit_qk(4, 0)

            pv_queue = []
            blocks = [(pos, t, p) for pos, t in enumerate((0, 1, 2, 3)) for p in range(4)]
            for n, (pos, t, p) in enumerate(blocks):
                state["pos"] = pos
                last = n == len(blocks) - 1
                for tt in range(t + 1):  # safety: deps normally already emitted
                    emit_qk(p, tt)
                    emit_qk(4 + p, tt)
                quota = list(W[n])
                # pt layout: [128, hh, chunk, 512]
                pt = ptpool.tile([P, 2, 16, 512], F16, tag="pt", name=f"pt_{p}_{t}")

                def group_hooks(quota=quota):
                    if pv_queue:
                        emit_pv(*pv_queue.pop(0))
                    if quota:
                        emit_item(quota.pop(0))

                # S^T + exp in groups of 2 chunks per head; diagonal chunks only
                # compute the causal-valid columns (stale psum prefix is bounded
                # old scores: exp'd then never consumed).
                for g in range(2 * t + 2):
                    psA = ps_s.tile([P, 2, 512], F32, tag="s", name=f"psA_{p}_{t}_{g}")
                    psB = ps_s.tile([P, 2, 512], F32, tag="s", name=f"psB_{p}_{t}_{g}")
                    for jj in (0, 1):
                        j = 2 * g + jj
                        q0 = 128 * (j - 4 * t) if j >= 4 * t else 0
                        for hh, ps in ((0, psA), (1, psB)):
                            nc.tensor.matmul(
                                ps[:, jj, q0:512],
                                lhsT=kt_sb[64 * hh : 64 * hh + 64, p, j * P : (j + 1) * P],
                                rhs=qt_sb[
                                    64 * hh : 64 * hh + 64,
                                    p,
                                    t * 512 + q0 : (t + 1) * 512,
                                ],
                                start=True,
                                stop=True,
                            )
                    for hh, ps in ((0, psA), (1, psB)):
                        if g == 2 * t + 1:
                            # fully-diagonal group: exp only the causal-valid
                            # suffixes (contiguous slices)
                            nc.scalar.activation(
                                pt[:, hh, 2 * g, 256:512],
                                ps[:, 0, 256:512],
                                mybir.ActivationFunctionType.Exp,
                                scale=0.125,
                            )
                            nc.scalar.activation(
                                pt[:, hh, 2 * g + 1, 384:512],
                                ps[:, 1, 384:512],
                                mybir.ActivationFunctionType.Exp,
                                scale=0.125,
                            )
                        else:
                            nc.scalar.activation(
                                pt[:, hh, 2 * g : 2 * g + 2, :],
                                ps[:],
                                mybir.ActivationFunctionType.Exp,
                                scale=0.125,
                            )
                    group_hooks()
                    if last and g >= 2 * t:
                        # final block: mask + PV inline per diagonal pair so the
                        # tail doesn't serialize after the last exp
                        for r in (0, 1) if g == 2 * t else (2, 3):
                            j = 4 * t + r
                            for hh in (0, 1):
                                blk = pt[:, hh, j, r * P : (r + 1) * P]
                                # DVE here: this mask sits on the tail critical
                                # chain and DVE is ~3x faster than GpSimd
                                nc.vector.tensor_mul(blk, blk, tri_sb[:])
                            emit_pv(p, t, pt, r, split_dma=(g == 2 * t + 1))
                while quota:
                    emit_item(quota.pop(0))
                while pv_queue:
                    emit_pv(*pv_queue.pop(0))
                if last:
                    continue
                # causal mask on diagonal 128x128 blocks (DVE: ~3x faster than
                # GpSimd and it has slack; next block's PV pops need these early)
                for hh in (0, 1):
                    for r in range(4):
                        j = 4 * t + r
                        blk = pt[:, hh, j, r * P : (r + 1) * P]
                        nc.vector.tensor_mul(blk, blk, tri_sb[:])
                pv_queue = [(p, t, pt, r) for r in range(4)]
            while pv_queue:
                emit_pv(*pv_queue.pop(0))

    nc.compile()
    return nc


def get_nc():
    if "nc" not in _cache:
        _cache["nc"] = _build()
    return _cache["nc"]


def _prep_core_inputs(x, W, b, bi, hg):
    h0 = hg * HL
    Wq = W[:, 0:D].reshape(D, H, HD)
    Wk = W[:, D : 2 * D].reshape(D, H, HD)
    Wv = W[:, 2 * D :].reshape(D, H, HD)
    bq = b[0:D].reshape(H, HD)
    bk = b[D : 2 * D].reshape(H, HD)
    bv = b[2 * D :].reshape(H, HD)

    # pair-major: pair p occupies cols [256p, 256p+256) as [Q pair | K pair]
    wqk = np.empty((D, 1024), np.float32)
    bqk = np.empty((P, 8), np.float32)
    for c in range(4):
        for half in range(2):
            h = h0 + 2 * c + half
            sl = slice(256 * c + half * HD, 256 * c + half * HD + HD)
            wqk[:, sl] = Wq[:, h]
            bqk[half * HD : (half + 1) * HD, c] = bq[h]
            sl = slice(256 * c + P + half * HD, 256 * c + P + half * HD + HD)
            wqk[:, sl] = Wk[:, h]
            bqk[half * HD : (half + 1) * HD, 4 + c] = bk[h]

    wv_aug = np.zeros((D, VW), np.float32)
    bv_aug = np.zeros((VW,), np.float32)
    for hl in range(HL):
        wv_aug[:, 65 * hl : 65 * hl + HD] = Wv[:, h0 + hl]
        bv_aug[65 * hl : 65 * hl + HD] = bv[h0 + hl]
        bv_aug[65 * hl + HD] = 1.0

    tri = np.triu(np.ones((P, P), np.float32))  # tri[k, q] = 1 where q >= k

    return {
        "x": np.ascontiguousarray(x[bi].astype(np.float16).T),
        "wqk": wqk.astype(np.float16),
        "wv": wv_aug.astype(np.float16),
        "bqk": bqk,
        "bv": np.broadcast_to(bv_aug.astype(np.float16), (P, VW)).copy(),
        "tri": tri.astype(np.float16),
    }


def make_in_maps(x, W_qkv, b_qkv):
    x = np.asarray(x, dtype=np.float32)
    W = np.asarray(W_qkv, dtype=np.float32)
    b = np.asarray(b_qkv, dtype=np.float32)
    return [_prep_core_inputs(x, W, b, i // 2, i % 2) for i in range(N_CORES)]


def assemble(results):
    out = np.empty((B, N, D), np.float32)
    for i in range(N_CORES):
        bi, hg = i // 2, i % 2
        out[bi, :, hg * 512 : (hg + 1) * 512] = results[i]["out"]
    return out


def run(x, W_qkv, b_qkv, trace=False, tmpdir=None):
    nc = get_nc()
    in_maps = make_in_maps(x, W_qkv, b_qkv)
    res = bass_utils.run_bass_kernel_spmd(
        nc, in_maps, core_ids=list(range(N_CORES)), trace=trace, tmpdir=tmpdir
    )
    return assemble(res.results), res


def kernel(x, W_qkv, b_qkv):
    out, _ = run(x, W_qkv, b_qkv)
    return out


# revision 6
# speedup vs baseline: 1.0541x; 1.0541x over previous
"""Causal multi-head attention (QKV projection + softmax(QK^T)V) on 8 TRN2 NeuronCores.

Problem: x[4,2048,1024] @ W_qkv[1024,3072] + b_qkv -> 16-head causal attention -> [4,2048,1024].

Sharding: core i = (batch bi=i//2, head-group hg=i%2). Each core handles 1 batch x 8 heads,
fully data/tensor-parallel (no collectives). Host pre-arranges per-core inputs (all matmul
operands fp16; accumulation f32 in PSUM):
  - x passed pre-transposed [1024, 2048] so the contraction dim lands on partitions with
    plain contiguous DMAs (no on-device transposes anywhere).
  - wqk [1024,1024] pair-major (pair p: Q cols at 256p, K at 256p+128), head-PAIR-stacked
    (64+64 rows) so QKV^T matmul output chunks are directly the [hd, n] stacked layout the
    attention stage consumes.
  - wv [1024,520]: V columns with per-head stride 65; col 65h+64 is a zero column, and
    the replicated bias tile bv has 1.0 there, so the "ones column" that makes the PV
    matmul accumulate softmax denominators (and b_v itself) ride the DVE PSUM->SBUF
    drain as a tensor_add -- no bias matmuls at all.
Device pipeline per core:
  QKV^T matmuls (Q^T pair-stacked, K^T pair-packed: head h occupies rows 64*(h%2) of its
  pair's slab, S^T runs as a 64-contraction matmul at base_partition 64*(h%2), which also
  row-tiles the two heads onto disjoint PE array row-groups) -> S^T = K Q^T per key-chunk
  with causal column trimming -> one ScalarE Exp(scale=1/8) per 2-chunk group, PSUM->SBUF
  fp16 = P^T -> causal tri-mask multiply on the 128x128 diagonal blocks only (DVE) ->
  PV matmuls accumulate [q, 64 cols + denominator] per q-block (both heads packed in one
  PSUM bank) -> reciprocal (DVE) * scale (DVE late / ScalarE early) epilogue ->
  DMA out [2048, 512] f32.
Scheduling: ScalarE exp (~160us) and TensorE (~190us) must overlap near-perfectly.
 - A dozen warm-up matmuls on memset scratch run first so the PE HAM clock-gate reaches
   K=8/8 (2.4 GHz) before real work; without them the DMA-gated trickle start keeps the
   PE at 1.2 GHz for the first ~20us of real matmuls.
 - Input DMA is spread over three rings so triggers issue in parallel: sync ring carries
   the latency-critical x-stripe-0 + wqk-pair-0 pieces per-chunk; the scalar ring (idle
   before the first softmax) carries batched x stripes 1-3; gpsimd SWDGE carries the
   rest (wqk pairs 1-3, wv, biases) -- ScalarE's instruction stream stays pure exps by
   the time softmax starts.
 - Attention runs stripe-major across head-pairs (t-major rounds). QKV matmul tiles are
   distributed across blocks by a build-time reverse-greedy quota scheduler: each block's
   spare PE time (ACT cost minus S^T/PV cost) is filled latest-first subject to each
   tile's consumer deadline, so the ACT-heavy late rounds keep the PE dense instead of
   exhausting the filler early.
 - Each stripe's PV matmuls are deferred into the next block's S^T/exp loop (inlined
   per-diagonal-chunk for the final block so the tail doesn't serialize).
"""

import numpy as np

import concourse.bass as bass
import concourse.tile as tile
from concourse import bacc, mybir
from concourse import bass_utils

F16 = mybir.dt.float16
F32 = mybir.dt.float32

B, N, D = 4, 2048, 1024
H = 16  # global heads
HD = 64
HL = 8  # heads per core
N_CORES = 8
P = 128
NT = N // P  # 16 token tiles
KC = D // P  # 8 contraction chunks
VW = HL * (HD + 1)  # 520
VH = VW // 2  # 260

_cache = {}


def _build():
    nc = bacc.Bacc("TRN2", target_bir_lowering=False, debug=False)

    x_d = nc.dram_tensor("x", [D, N], F16, kind="ExternalInput").ap()  # x^T, host-transposed
    wqk_d = nc.dram_tensor("wqk", [D, 1024], F16, kind="ExternalInput").ap()
    wv_d = nc.dram_tensor("wv", [D, VW], F16, kind="ExternalInput").ap()
    bqk_d = nc.dram_tensor("bqk", [P, 8], F32, kind="ExternalInput").ap()
    bv_d = nc.dram_tensor("bv", [P, VW], F16, kind="ExternalInput").ap()
    tri_d = nc.dram_tensor("tri", [P, P], F16, kind="ExternalInput").ap()
    out_d = nc.dram_tensor("out", [N, HL * HD], F32, kind="ExternalOutput").ap()

    wqk_r = wqk_d.rearrange("(k p) n -> p k n", p=P)
    wv_r = wv_d.rearrange("(k p) n -> p k n", p=P)
    x_r = x_d.rearrange("(k p) n -> p k n", p=P)

    with tile.TileContext(nc) as tc:
        with (
            tc.tile_pool(name="const", bufs=1) as cpool,
            tc.tile_pool(name="pt", bufs=2) as ptpool,
            tc.tile_pool(name="opair", bufs=6) as oppool,
            tc.tile_pool(name="misc", bufs=6) as mpool,
            tc.tile_pool(name="ps_mm", bufs=2, space="PSUM") as ps_mm,
            tc.tile_pool(name="ps_s", bufs=2, space="PSUM") as ps_s,
            tc.tile_pool(name="ps_o", bufs=2, space="PSUM") as ps_o,
        ):
            # ---- constants / inputs to SBUF ----
            xt_sb = cpool.tile([P, KC, N], F16, name="xt_sb")  # x^T, 8 chunks of [128, 2048]
            wqk_sb = cpool.tile([P, KC, 1024], F16, name="wqk_sb")
            wv_sb = cpool.tile([P, KC, VW], F16, name="wv_sb")
            bqk_sb = cpool.tile([P, 8], F32, name="bqk_sb")
            bv_sb = cpool.tile([P, VW], F16, name="bv_sb")  # b_v (+ones col) replicated
            tri_sb = cpool.tile([P, P], F16, name="tri_sb")
            qt_sb = cpool.tile([P, 4, N], F16, name="qt_sb")  # Q^T pair-stacked
            # K^T pair-packed: pair pr's slab holds head 2pr K^T in rows 0:64 and head
            # 2pr+1 in rows 64:128. S^T runs as a 64-row contraction at base_partition
            # 64*hh -- no zero padding, no memset, and the two heads land on disjoint
            # PE row-groups so their LDWEIGHTS/MATMULs overlap in the array.
            kt_sb = cpool.tile([P, 4, N], F16, name="kt_sb")
            v_sb = cpool.tile([P, NT, VW], F16, name="v_sb")
            wu_sb = cpool.tile([P, 640], F16, name="wu_sb")  # PE warm-up scratch

            # PE warm-up: the HAM clock gate un-throttles (1.2 -> 2.4 GHz) only after
            # ~3.4us of sustained PE activity. Burn that window on scratch matmuls while
            # the input DMAs are in flight, so real matmuls start at full clock.
            nc.gpsimd.memset(wu_sb[:], 0.0)
            psw = ps_mm.tile([P, 512], F32, tag="mm", name="ps_warm")
            for i in range(12):
                nc.tensor.matmul(
                    psw[:],
                    lhsT=wu_sb[:, 0:P],
                    rhs=wu_sb[:, P : P + 512],
                    start=(i == 0),
                    stop=(i == 11),
                )

            # Preload the exp table set (~2.7us) before the first real softmax exp, so
            # it doesn't pay ACT_TABLE_LOAD.
            warm = mpool.tile([1, 8], F32, tag="warm", name="warm")
            nc.gpsimd.memset(warm[:], 0.0)
            nc.scalar.activation(warm[:], warm[:], mybir.ActivationFunctionType.Exp)

            # ---- input DMA: one ring (sync), ordered by first-use deadline ----
            # Batched triggers (vs 59 per-chunk ones) so serialization at ~620ns per
            # trigger doesn't gate the start; ordering keeps the critical first-tile
            # bytes (x stripe-0 + wqk pair-0) exclusive on the DMA engines, then bulk
            # follows in deadline order.
            nc.sync.dma_start(bqk_sb[:], bqk_d)
            nc.sync.dma_start(xt_sb[:, 0:4, 0:512], x_r[:, 0:4, 0:512])
            nc.sync.dma_start(wqk_sb[:, 0:4, 0:256], wqk_r[:, 0:4, 0:256])
            nc.sync.dma_start(xt_sb[:, 4:8, 0:512], x_r[:, 4:8, 0:512])
            nc.sync.dma_start(wqk_sb[:, 4:8, 0:256], wqk_r[:, 4:8, 0:256])
            nc.sync.dma_start(tri_sb[:], tri_d)
            nc.sync.dma_start(wv_sb[:, :, :], wv_r[:, :, :])
            nc.sync.dma_start(wqk_sb[:, :, 256:512], wqk_r[:, :, 256:512])
            nc.sync.dma_start(bv_sb[:], bv_d)
            nc.sync.dma_start(
                xt_sb[:, :, 512:1024], x_r[:, :, 512:1024]
            )
            nc.sync.dma_start(wqk_sb[:, :, 512:768], wqk_r[:, :, 512:768])
            nc.sync.dma_start(wqk_sb[:, :, 768:1024], wqk_r[:, :, 768:1024])
            nc.sync.dma_start(
                xt_sb[:, :, 1024:1536], x_r[:, :, 1024:1536]
            )
            nc.sync.dma_start(
                xt_sb[:, :, 1536:2048], x_r[:, :, 1536:2048]
            )

            done_qk = set()
            done_v = set()

            def emit_qk(c, tt):
                """QKV^T matmul tile for col-chunk c, token stripe tt."""
                if (c, tt) in done_qk:
                    return
                done_qk.add((c, tt))
                pr = c % 4
                pq = ps_mm.tile([P, 512], F32, tag="mm", name=f"pq_{c}_{tt}")
                col0 = 256 * (c % 4) + (0 if c < 4 else 128)
                for k in range(KC):
                    nc.tensor.matmul(
                        pq[:],
                        lhsT=wqk_sb[:, k, col0 : col0 + P],
                        rhs=xt_sb[:, k, tt * 512 : (tt + 1) * 512],
                        start=(k == 0),
                        stop=(k == KC - 1),
                    )

                def badd(out, in_, b):
                    nc.vector.tensor_scalar_add(out, in_, b)

                if c < 4:
                    badd(
                        qt_sb[:, pr, tt * 512 : (tt + 1) * 512], pq[:], bqk_sb[:, c : c + 1]
                    )
                else:
                    for hh in (0, 1):
                        rows = slice(64 * hh, 64 * hh + 64)
                        badd(
                            kt_sb[rows, pr, tt * 512 : (tt + 1) * 512],
                            pq[rows, :],
                            bqk_sb[rows, c : c + 1],
                        )

            def emit_v(j, half):
                """V (augmented) for token tile j, half (260 cols each)."""
                if (j, half) in done_v:
                    return
                done_v.add((j, half))
                pv = ps_mm.tile([P, VH], F32, tag="mm", name=f"pv_{j}_{half}")
                for k in range(KC):
                    nc.tensor.matmul(
                        pv[:],
                        lhsT=xt_sb[:, k, j * P : (j + 1) * P],
                        rhs=wv_sb[:, k, half * VH : (half + 1) * VH],
                        start=(k == 0),
                        stop=(k == KC - 1),
                    )
                # bias (and the denominator ones-column) ride the PSUM->SBUF drain
                nc.vector.tensor_add(
                    v_sb[:, j, half * VH : (half + 1) * VH],
                    pv[:],
                    bv_sb[:, half * VH : (half + 1) * VH],
                )

            # ---- build-time quota scheduler for QKV filler ----
            # blocks processed t-major: n = 4*t + p
            n_blocks = 16

            def act_cost(t):
                return (2 * t + 1) * 2086 + 1400

            def pe_s_cost(t):
                # the two heads' S^T matmuls run concurrently (disjoint row groups),
                # so per-group wall is the single-head column count
                return (2048 * t + 1280) * 0.4167 + (2 * t + 2) * 120

            def pv_cost(t):
                return 2 * (16 * t + 10) * 34.0

            QK_COST = 8 * 512 * 0.4167 + 8 * 15
            V_COST = 8 * 260 * 0.4167 + 8 * 15

            # items: (kind, a, b, deadline_block, earliest_block, cost)
            items = []
            for tt in range(4):
                for pr in range(4):
                    if (pr, tt) == (0, 0):
                        continue  # prologue
                    dl = 4 * tt + pr
                    for c in (pr, 4 + pr):
                        items.append(["qk", c, tt, dl, 2 * tt, QK_COST])
            for tj in range(4):
                for j in range(4 * tj, 4 * tj + 4):
                    for half in (0, 1):
                        dl = 4 * tj + 2 * half + 1
                        items.append(["v", j, half, dl, 2 * tj, V_COST])

            cap = []
            for n in range(n_blocks):
                t = n // 4
                c = act_cost(t) - pe_s_cost(t)
                if n >= 1:
                    c -= pv_cost((n - 1) // 4)
                if n == n_blocks - 1:
                    c -= pv_cost(3)  # own inline PV
                cap.append(max(0.0, c))

            # Forward greedy: place items as EARLY as capacity allows (ACT has its
            # structural slack in the small early rounds; late rounds must run with
            # ScalarE saturated and no extra PE work between S^T groups).
            W = [[] for _ in range(n_blocks)]
            remaining = list(items)
            for n in range(n_blocks):
                room = cap[n]
                # mandatory: last chance for items with deadline n+1
                musts = [it for it in remaining if it[3] == n + 1]
                for it in musts:
                    W[n].append(it)
                    room -= it[5]
                    remaining.remove(it)
                elig = [it for it in remaining if it[4] <= n]
                elig.sort(key=lambda it: it[3])  # earliest deadline first
                for it in elig:
                    if room <= 0:
                        break
                    W[n].append(it)
                    room -= it[5]
                    remaining.remove(it)
            for n in range(n_blocks):
                W[n].sort(key=lambda it: it[3])  # urgent first within a block

            def emit_item(it):
                if it[0] == "qk":
                    emit_qk(it[1], it[2])
                else:
                    emit_v(it[1], it[2])

            state = {}

            def emit_pv_half(p, t, pt, r, hh, ctx, split_dma=False):
                """One head's PV chain for q-block i = 4t+r; epilogue+DMA after hh=1.
                Both heads' accumulators share one PSUM bank ([128, 2, 65])."""
                i = 4 * t + r
                hl = 2 * p + hh
                for j in range(i + 1):  # safety: deps normally already emitted
                    emit_v(j, hl // 4)
                if hh == 0:
                    ctx["opair"] = oppool.tile([P, P], F32, tag="op", name=f"op_{p}_{i}")
                    ctx["po"] = po = ps_o.tile([P, 2, 65], F32, tag="o", name=f"po_{p}_{i}")
                else:
                    po = ctx["po"]
                for j in range(i + 1):
                    nc.tensor.matmul(
                        po[:, hh, :],
                        lhsT=pt[:, hh, j, r * P : (r + 1) * P],
                        rhs=v_sb[:, j, 65 * hl : 65 * hl + 65],
                        start=(j == 0),
                        stop=(j == i),
                    )
                if hh == 0:
                    return
                opair = ctx["opair"]
                rc = mpool.tile([P, 2], F32, tag="rc", name=f"rc_{p}_{i}")
                nc.vector.reciprocal(rc[:], po[:, :, 64])
                for h2 in (0, 1):
                    if state.get("pos", 0) == 0:
                        # early rounds are PE/DVE-bound and ScalarE has slack
                        nc.scalar.mul(
                            opair[:, 64 * h2 : 64 * h2 + 64],
                            po[:, h2, 0:64],
                            rc[:, h2 : h2 + 1],
                        )
                    else:
                        nc.vector.tensor_scalar_mul(
                            opair[:, 64 * h2 : 64 * h2 + 64], po[:, h2, 0:64], rc[:, h2 : h2 + 1]
                        )
                    if split_dma:
                        nc.sync.dma_start(
                            out_d[i * P : (i + 1) * P, p * P + 64 * h2 : p * P + 64 * h2 + 64],
                            opair[:, 64 * h2 : 64 * h2 + 64],
                        )
                if not split_dma:
                    nc.sync.dma_start(out_d[i * P : (i + 1) * P, p * P : (p + 1) * P], opair[:])

            def emit_pv(p, t, pt, r, split_dma=False):
                ctx = {}
                emit_pv_half(p, t, pt, r, 0, ctx, split_dma)
                emit_pv_half(p, t, pt, r, 1, ctx, split_dma)

            # Prologue: the first block's own QK stripes so S^T (0,0) can start ASAP.
            emit_qk(0, 0)
            emit_qk(4, 0)

            pv_queue = []
            blocks = [(pos, t, p) for pos, t in enumerate((0, 1, 2, 3)) for p in range(4)]
            for n, (pos, t, p) in enumerate(blocks):
                state["pos"] = pos
                last = n == len(blocks) - 1
                for tt in range(t + 1):  # safety: deps normally already emitted
                    emit_qk(p, tt)
                    emit_qk(4 + p, tt)
                quota = list(W[n])
                # pt layout: [128, hh, chunk, 512]
                pt = ptpool.tile([P, 2, 16, 512], F16, tag="pt", name=f"pt_{p}_{t}")

                def group_hooks(quota=quota):
                    if pv_queue:
                        emit_pv(*pv_queue.pop(0))
                    if quota:
                        emit_item(quota.pop(0))

                # S^T + exp in groups of 2 chunks per head; diagonal chunks only
                # compute the causal-valid columns (stale psum prefix is bounded
                # old scores: exp'd then never consumed).
                for g in range(2 * t + 2):
                    psA = ps_s.tile([P, 2, 512], F32, tag="s", name=f"psA_{p}_{t}_{g}")
                    psB = ps_s.tile([P, 2, 512], F32, tag="s", name=f"psB_{p}_{t}_{g}")
                    for jj in (0, 1):
                        j = 2 * g + jj
                        q0 = 128 * (j - 4 * t) if j >= 4 * t else 0
                        for hh, ps in ((0, psA), (1, psB)):
                            nc.tensor.matmul(
                                ps[:, jj, q0:512],
                                lhsT=kt_sb[64 * hh : 64 * hh + 64, p, j * P : (j + 1) * P],
                                rhs=qt_sb[
                                    64 * hh : 64 * hh + 64,
                                    p,
                                    t * 512 + q0 : (t + 1) * 512,
                                ],
                                start=True,
                                stop=True,
                            )
                    for hh, ps in ((0, psA), (1, psB)):
                        if g == 2 * t + 1:
                            # fully-diagonal group: exp only the causal-valid
                            # suffixes (contiguous slices)
                            nc.scalar.activation(
                                pt[:, hh, 2 * g, 256:512],
                                ps[:, 0, 256:512],
                                mybir.ActivationFunctionType.Exp,
                                scale=0.125,
                            )
                            nc.scalar.activation(
                                pt[:, hh, 2 * g + 1, 384:512],
                                ps[:, 1, 384:512],
                                mybir.ActivationFunctionType.Exp,
                                scale=0.125,
                            )
                        else:
                            nc.scalar.activation(
                                pt[:, hh, 2 * g : 2 * g + 2, :],
                                ps[:],
                                mybir.ActivationFunctionType.Exp,
                                scale=0.125,
                            )
                    group_hooks()
                    if last and g >= 2 * t:
                        # final block: mask + PV inline per diagonal pair so the
                        # tail doesn't serialize after the last exp
                        for r in (0, 1) if g == 2 * t else (2, 3):
                            j = 4 * t + r
                            for hh in (0, 1):
                                blk = pt[:, hh, j, r * P : (r + 1) * P]
                                # DVE here: this mask sits on the tail critical
                                # chain and DVE is ~3x faster than GpSimd
                                nc.vector.tensor_mul(blk, blk, tri_sb[:])
                            emit_pv(p, t, pt, r, split_dma=(g == 2 * t + 1))
                while quota:
                    emit_item(quota.pop(0))
                while pv_queue:
                    emit_pv(*pv_queue.pop(0))
                if last:
                    continue
                # causal mask on diagonal 128x128 blocks (DVE: ~3x faster than
                # GpSimd and it has slack; next block's PV pops need these early)
                for hh in (0, 1):
                    for r in range(4):
                        j = 4 * t + r
                        blk = pt[:, hh, j, r * P : (r + 1) * P]
                        nc.vector.tensor_mul(blk, blk, tri_sb[:])
                pv_queue = [(p, t, pt, r) for r in range(4)]
            while pv_queue:
                emit_pv(*pv_queue.pop(0))

    nc.compile()
    return nc


def get_nc():
    if "nc" not in _cache:
        _cache["nc"] = _build()
    return _cache["nc"]


def _prep_core_inputs(x, W, b, bi, hg):
    h0 = hg * HL
    Wq = W[:, 0:D].reshape(D, H, HD)
    Wk = W[:, D : 2 * D].reshape(D, H, HD)
    Wv = W[:, 2 * D :].reshape(D, H, HD)
    bq = b[0:D].reshape(H, HD)
    bk = b[D : 2 * D].reshape(H, HD)
    bv = b[2 * D :].reshape(H, HD)

    # pair-major: pair p occupies cols [256p, 256p+256) as [Q pair | K pair]
    wqk = np.empty((D, 1024), np.float32)
    bqk = np.empty((P, 8), np.float32)
    for c in range(4):
        for half in range(2):
            h = h0 + 2 * c + half
            sl = slice(256 * c + half * HD, 256 * c + half * HD + HD)
            wqk[:, sl] = Wq[:, h]
            bqk[half * HD : (half + 1) * HD, c] = bq[h]
            sl = slice(256 * c + P + half * HD, 256 * c + P + half * HD + HD)
            wqk[:, sl] = Wk[:, h]
            bqk[half * HD : (half + 1) * HD, 4 + c] = bk[h]

    wv_aug = np.zeros((D, VW), np.float32)
    bv_aug = np.zeros((VW,), np.float32)
    for hl in range(HL):
        wv_aug[:, 65 * hl : 65 * hl + HD] = Wv[:, h0 + hl]
        bv_aug[65 * hl : 65 * hl + HD] = bv[h0 + hl]
        bv_aug[65 * hl + HD] = 1.0

    tri = np.triu(np.ones((P, P), np.float32))  # tri[k, q] = 1 where q >= k

    return {
        "x": np.ascontiguousarray(x[bi].astype(np.float16).T),
        "wqk": wqk.astype(np.float16),
        "wv": wv_aug.astype(np.float16),
        "bqk": bqk,
        "bv": np.broadcast_to(bv_aug.astype(np.float16), (P, VW)).copy(),
        "tri": tri.astype(np.float16),
    }


def make_in_maps(x, W_qkv, b_qkv):
    x = np.asarray(x, dtype=np.float32)
    W = np.asarray(W_qkv, dtype=np.float32)
    b = np.asarray(b_qkv, dtype=np.float32)
    return [_prep_core_inputs(x, W, b, i // 2, i % 2) for i in range(N_CORES)]


def assemble(results):
    out = np.empty((B, N, D), np.float32)
    for i in range(N_CORES):
        bi, hg = i // 2, i % 2
        out[bi, :, hg * 512 : (hg + 1) * 512] = results[i]["out"]
    return out


def run(x, W_qkv, b_qkv, trace=False, tmpdir=None):
    nc = get_nc()
    in_maps = make_in_maps(x, W_qkv, b_qkv)
    res = bass_utils.run_bass_kernel_spmd(
        nc, in_maps, core_ids=list(range(N_CORES)), trace=trace, tmpdir=tmpdir
    )
    return assemble(res.results), res


def kernel(x, W_qkv, b_qkv):
    out, _ = run(x, W_qkv, b_qkv)
    return out
